# revision 51
# baseline (speedup 1.0000x reference)
"""Trainium2 Bass kernel for a causal attention block (B=2, T=2048, E=2048,
16 heads, head_dim=128, interleaved RoPE).

Sharding: data-parallel over batch (2) x tensor-parallel over heads (4 per
core) = 8 NeuronCores. Each core computes QKV projection for its 4 heads,
RoPE, causal SDPA, and a partial output projection (row-sharded W_out); the
host sums the 4 TP partials per batch element.

Single software-pipelined instruction stream (fp16 matmuls, fp32 PSUM):
  - warmup matmuls on a memset tile ramp the PE p-state while the first
    DMAs land (the clock drops back to mid p-state after any PE idle, so
    startup stalls cost ~2x their face value).
  - startup transfers are striped across the sync and scalar trigger
    queues (~150-190GB/s each, ~380GB/s aggregate; the gpsimd queue is
    25x slower) in exact consumption order, and the first two QK units
    run block-staggered chunk-interleaved so each freshly-landed x chunk
    feeds 1024 output rows — the PE consumption rate matches the DMA
    supply rate instead of outrunning it 2:1.
  - segment g in 0..3 runs the QKV projection for token slab g (QT/KT
    transposed + RoPE'd on eviction, V natural) interleaved with SDPA for
    query tile g-1; causality means tile g-1 only needs slabs <= g-1, and
    the dependency-free projection matmuls keep the PE busy while ACT/DVE
    drain the softmax chain.  Slabs prefetch a full segment early, split
    across both fast queues.
  - SDPA diagonal blocks are column-restricted (queries below the diagonal
    are never computed), the softmax denominator tree runs in fp16 (DVE 4x
    mode) + a ones-matmul partition reduction, and the output projection
    for query tiles 0..2 interleaves into the tq=3 SDPA round so its
    output DMAs drain during compute.
fp8 was evaluated and rejected: DoubleRow matmuls measure 2x fp16 on
real hw (not the cost model's 4x), so the accuracy-preserving hi/lo
3-pass split is slower than fp16, and raw 1-pass fp8 fails the 2e-2
gate (4.4e-2 measured in simulation).
"""

import sys
from contextlib import ExitStack

sys.path.insert(0, "/opt/trn_rl_repo")

import numpy as np

import bass_rust
import concourse.bacc as bacc
import concourse.mybir as mybir
from concourse.alu_op_type import AluOpType
from concourse import tile
from concourse import bass_utils

B, T, E = 2, 2048, 2048
N_HEAD = 16
D = E // N_HEAD            # 128
THETA = 10000.0
N_CORES = 8
TP = 4                     # tensor-parallel degree (heads)
HPC = N_HEAD // TP         # heads per core = 4
FL = HPC * D               # local head width = 512
EC = E // 128              # 16 contraction chunks
TQ = 512                   # query tile (free dim)
NTQ = T // TQ              # 4
NTK = T // 128             # 16
N_WARM = 17                # PE p-state warmup matmuls

F32 = mybir.dt.float32
F32R = mybir.dt.float32r
F16 = mybir.dt.float16
EXP = mybir.ActivationFunctionType.Exp
SCALE = 1.0 / np.sqrt(D)

_compiled = None
_last_in_maps = None


def _build():
    nc = bacc.Bacc("TRN2", target_bir_lowering=False)

    xT = nc.dram_tensor("xT", (NTQ, 128, EC * TQ), F16, kind="ExternalInput")
    wqk = nc.dram_tensor("wqk", (2 * HPC, 128, EC * 128), F16, kind="ExternalInput")
    wv = nc.dram_tensor("wv", (128, EC * FL), F16, kind="ExternalInput")
    wout = nc.dram_tensor("wout", (HPC, 128, E), F16, kind="ExternalInput")
    csx = nc.dram_tensor("csx", (128, T), F16, kind="ExternalInput")
    mask1 = nc.dram_tensor("mask1", (128, 128), F16, kind="ExternalInput")
    ones_m = nc.dram_tensor("ones_m", (128, 128), F16, kind="ExternalInput")
    out = nc.dram_tensor("out", (T, E), F16, kind="ExternalOutput")

    with tile.TileContext(nc) as tc, nc.allow_low_precision(
        reason="fp16 matmul inputs / fp16 softmax stats are intentional"
    ), tc.tile_pool(name="const", bufs=1) as const, \
         tc.tile_pool(name="wo_p", bufs=1) as wo_p, \
         tc.tile_pool(name="qkt_p", bufs=1) as qkt_p, \
         tc.tile_pool(name="v_p", bufs=1) as v_p, \
         tc.tile_pool(name="yt_p", bufs=1) as yt_p, \
         tc.tile_pool(name="es_p", bufs=12) as es_p, \
         tc.tile_pool(name="pair_p", bufs=4) as pair_p, \
         tc.tile_pool(name="dn_p", bufs=1) as dn_p, \
         tc.tile_pool(name="mm_ps", bufs=2, space="PSUM") as mm_ps, \
         tc.tile_pool(name="sc_ps", bufs=4, space="PSUM") as sc_ps, \
         tc.tile_pool(name="y_ps", bufs=2, space="PSUM") as y_ps:

        # phase-1 pools: closed before the final projection segment to free
        # SBUF for the output-eviction pool (stack allocation is LIFO)
        p1_stack = ExitStack()
        wqk_p = p1_stack.enter_context(tc.tile_pool(name="wqk_p", bufs=1))
        wv_p = p1_stack.enter_context(tc.tile_pool(name="wv_p", bufs=1))
        xt_p = p1_stack.enter_context(tc.tile_pool(name="xt_p", bufs=2))
        rope_t = p1_stack.enter_context(tc.tile_pool(name="rope_t", bufs=3))

        cs_sb = const.tile([128, T], F16, tag="cs")    # [cos; sin]
        csd_sb = const.tile([128, T], F16, tag="csd")  # [sin; cos]
        mask_sb = const.tile([128, 128], F16, tag="mask")
        onem = const.tile([128, 128], F16, tag="onem")
        warm_sb = const.tile([128, TQ], F16, tag="warm")

        # wqk stored f8-major: one [128, EC*128] tile per 128-wide qk block so
        # the first projection unit is gated by 0.5MB of weights, not 4MB
        wqk_sb = [wqk_p.tile([128, EC * 128], F16, tag=f"wqk{f}", name=f"wqk_sb{f}")
                  for f in range(2 * HPC)]
        wv_all = wv_p.tile([128, EC * FL], F16, tag="wv")
        wo_sb = [wo_p.tile([128, E], F16, tag=f"wo{h}", name=f"wo_sb{h}")
                 for h in range(HPC)]

        # resident intermediates: QT/KT (transposed, de-interleaved, RoPE'd),
        # V (natural layout), normalized attention outputs
        qkt_sb = [[qkt_p.tile([128, TQ], F16, tag=f"qkt{f}_{t}", name=f"qkt_sb{f}_{t}")
                   for t in range(NTQ)] for f in range(2 * HPC)]
        v_sb = [v_p.tile([128, FL], F16, tag=f"v{t}", name=f"v_sb{t}")
                for t in range(NTK)]
        yt_sb = [[yt_p.tile([128, TQ], F16, tag=f"yt{h}_{t}", name=f"yt_sb{h}_{t}")
                  for t in range(NTQ)] for h in range(HPC)]

        # ---------------- DMA + warmup ----------------
        xt_sb = {}

        def dma_slab(t4):
            # the trigger is WAR-gated on the xt ring, so when it finally
            # fires the transfer must finish quickly.  Slab 1 rides as two
            # halves (its transfer shares the startup window, where extra
            # triggers interfere); slabs 2-3 fire mid-kernel on idle
            # queues, so they stripe as chunk-ordered quarters and the
            # first-consumed chunks land at ~WAR+2.7us instead of +5.3.
            xt = xt_p.tile([128, EC * TQ], F16, tag="xt", name=f"xt_{t4}")
            n = 2 if t4 < 2 else 4
            q = EC * TQ // n
            for i in range(n):
                eng = nc.sync if i % 2 == 0 else nc.scalar
                eng.dma_start(xt[:, i * q:(i + 1) * q],
                              xT[t4, :, i * q:(i + 1) * q])
            xt_sb[t4] = xt

        # PE p-state warmup on a memset tile (no DMA dependency)
        nc.gpsimd.memset(warm_sb[:], 1.0)

        # startup DMA: slab 0 rides the sync queue as 4-chunk group DMAs
        # (4KB/partition lines; 16 small transfers would pay ~600ns trigger
        # cost each and the queue's ~190GB/s issue rate); weights ride the
        # scalar queue in parallel (the 16 shared DMA engines give
        # ~380GB/s aggregate); slab 1 prefetches behind slab 0.
        # NB: the gpsimd-triggered DMA queue moves ~1.5GB/s/engine (25x
        # slower than sync/scalar queues) — only tiny constants go there.
        xt0 = xt_p.tile([128, EC * TQ], F16, tag="xt", name="xt_0")
        xt_sb[0] = xt0

        # each fast queue serializes its transfers at ~150-190GB/s from
        # ~8.6us, so arrivals are predictable: stripe x groups and weights
        # across sync/scalar in exact pair-consumption order.  The pair
        # unit consumes a fresh 4-chunk group every ~1.7us, matching the
        # striped supply, so the PE never stalls (and never drops p-state).
        # sync:   g0, wqk1, g2, cs, wqk4-7, slab1a
        # scalar: wqk0, g1, g3, wqk2, wqk3, (inj:) wv, slab1b
        def xgrp(e4):
            return (xt0[:, e4 * TQ:(e4 + 4) * TQ],
                    xT[0, :, e4 * TQ:(e4 + 4) * TQ])

        nc.sync.dma_start(*xgrp(0))
        nc.sync.dma_start(wqk_sb[1][:], wqk[1])
        nc.sync.dma_start(*xgrp(8))
        nc.sync.dma_start(wqk_sb[2][:], wqk[2])
        nc.sync.dma_start(cs_sb[:], csx[:])
        for f in (4, 6, 7):
            nc.sync.dma_start(wqk_sb[f][:], wqk[f])
        nc.scalar.dma_start(wqk_sb[0][:], wqk[0])
        nc.scalar.dma_start(*xgrp(4))
        nc.scalar.dma_start(*xgrp(12))
        nc.scalar.dma_start(wqk_sb[3][:], wqk[3])
        # wqk5 rides scalar (idle after wqk3) so the mid-phase weights all
        # arrive with ~3.4us of margin instead of racing consumption
        nc.scalar.dma_start(wqk_sb[5][:], wqk[5])
        # preload the ACT function tables (1.3us) during the DMA window so
        # the first qk16 eviction isn't serialized behind the table load
        # (after the triggers: the scalar engine must not delay them)
        actw = const.tile([128, 8], F16, tag="actw")
        nc.scalar.activation(actw[:], warm_sb[:, 0:8], EXP, scale=0.125)
        nc.scalar.copy(actw[:], warm_sb[:, 0:8])
        nc.gpsimd.dma_start(mask_sb[:], mask1[:])
        nc.gpsimd.dma_start(onem[:], ones_m[:])
        # csd = [sin; cos] is cs with halves swapped: derive on-chip
        nc.vector.tensor_copy(csd_sb[0:64, :], cs_sb[64:128, :])
        nc.vector.tensor_copy(csd_sb[64:128, :], cs_sb[0:64, :])
        wps = sc_ps.tile([128, TQ], F32, tag="sc", name="warm_ps")
        for i in range(N_WARM):
            nc.tensor.matmul(wps[:], warm_sb[:, 0:128], warm_sb[:],
                             start=True, stop=True, skip_group_check=True)

        # ---------------- unit builders ----------------
        qk16_sb = {}

        def qk_unit_mm(t4, f8):
            def f():
                ps = mm_ps.tile([128, TQ], F32, tag="mm", name=f"qkps_{t4}_{f8}")
                for e in range(EC):
                    nc.tensor.matmul(
                        ps[:], wqk_sb[f8][:, e * 128:(e + 1) * 128],
                        xt_sb[t4][:, e * TQ:(e + 1) * TQ],
                        start=(e == 0), stop=(e == EC - 1),
                        skip_group_check=True,
                    )
                qk16 = rope_t.tile([128, TQ], F16, tag="qk16",
                                   name=f"qk16_{t4}_{f8}")
                nc.scalar.copy(qk16[:], ps[:])
                qk16_sb[(t4, f8)] = qk16
            return (16 * 216, f)

        def qk_pair_mm(t4):
            # units f8=0,1 interleaved chunk-wise: each freshly-DMA'd x
            # chunk feeds 1024 output rows instead of 512, halving the
            # startup demand rate so the slab-0 transfers keep pace with
            # the PE (a stalled PE also drops back to mid p-state, so
            # startup stalls cost ~2x their face value).  psb lives in the
            # sc_ps ring so the next unit's mm psum has no WAR on us.
            def f():
                psa = mm_ps.tile([128, TQ], F32, tag="mm", name=f"qkps_{t4}_0")
                psb = sc_ps.tile([128, TQ], F32, tag="sc", name=f"qkps_{t4}_1")

                def mm(ps, f8, e):
                    nc.tensor.matmul(
                        ps[:], wqk_sb[f8][:, e * 128:(e + 1) * 128],
                        xt_sb[t4][:, e * TQ:(e + 1) * TQ],
                        start=(e == 0), stop=(e == EC - 1),
                        skip_group_check=True,
                    )

                # block-staggered: b trails a by one 4-chunk group, so
                # wqk1 is needed ~0.85us after start and each fresh group
                # feeds 8 matmuls (a once, b once)
                for e4 in range(0, EC, 4):
                    for e in range(e4, e4 + 4):
                        mm(psa, 0, e)
                    if e4 == EC - 4:
                        qk16a = rope_t.tile([128, TQ], F16, tag="qk16",
                                            name=f"qk16_{t4}_0")
                        nc.scalar.copy(qk16a[:], psa[:])
                        qk16_sb[(t4, 0)] = qk16a
                    for e in range(e4, e4 + 4):
                        mm(psb, 1, e)
                qk16b = rope_t.tile([128, TQ], F16, tag="qk16",
                                    name=f"qk16_{t4}_1")
                nc.scalar.copy(qk16b[:], psb[:])
                qk16_sb[(t4, 1)] = qk16b
            return (2 * 16 * 216, f)

        def qk_unit_rope(t4, f8):
            # separate weave unit: keeps the DVE queue free for sdpa masks
            # between rope batches (rope results are only needed next segment)
            def f():
                ts4 = slice(t4 * TQ, (t4 + 1) * TQ)
                dst = qkt_sb[f8][t4]
                qk16 = qk16_sb[(t4, f8)]
                t_a = rope_t.tile([64, TQ], F16, tag="ta")
                t_b = rope_t.tile([64, TQ], F16, tag="tb")
                nc.vector.tensor_tensor(t_a[:], qk16[0:64, :], cs_sb[0:64, ts4], op=AluOpType.mult)
                nc.vector.tensor_tensor(t_b[:], qk16[64:128, :], cs_sb[64:128, ts4], op=AluOpType.mult)
                nc.vector.tensor_tensor(dst[0:64, :], t_a[:], t_b[:], op=AluOpType.subtract)
                t_c = rope_t.tile([64, TQ], F16, tag="tc")
                t_d = rope_t.tile([64, TQ], F16, tag="td")
                nc.vector.tensor_tensor(t_c[:], qk16[0:64, :], csd_sb[0:64, ts4], op=AluOpType.mult)
                nc.vector.tensor_tensor(t_d[:], qk16[64:128, :], csd_sb[64:128, ts4], op=AluOpType.mult)
                nc.vector.tensor_tensor(dst[64:128, :], t_c[:], t_d[:], op=AluOpType.add)
            return (300, f)

        def v_unit(t4, i):
            def f():
                tk = 4 * t4 + i
                ps = mm_ps.tile([128, FL], F32, tag="mm", name=f"vps_{tk}")
                for e in range(EC):
                    nc.tensor.matmul(
                        ps[:], xt_sb[t4][:, e * TQ + i * 128:e * TQ + (i + 1) * 128],
                        wv_all[:, e * FL:(e + 1) * FL],
                        start=(e == 0), stop=(e == EC - 1),
                        skip_group_check=True,
                    )
                nc.scalar.copy(v_sb[tk][:], ps[:])
            return (16 * 216, f)

        def p1_units(t4):
            if t4 == 0:
                mm = [qk_pair_mm(0)] + \
                     [qk_unit_mm(0, f8) for f8 in range(2, 2 * HPC)] + \
                     [v_unit(0, i) for i in range(4)]
            else:
                mm = [qk_unit_mm(t4, f8) for f8 in range(2 * HPC)] + \
                     [v_unit(t4, i) for i in range(4)]
            rp = [qk_unit_rope(t4, f8) for f8 in range(2 * HPC)]
            units = []
            for i, u in enumerate(mm):
                units.append(u)
                if i >= 1 and i - 1 < len(rp):
                    units.append(rp[i - 1])
            units.extend(rp[len(mm) - 1:])
            return units

        def sdpa_units(tq):
            units = []
            head_units = []
            for h in range(HPC):
                st = {"fulls": [], "rag": [], "yps": None, "es": {}}
                nblk = 4 * tq + 4

                def blk_a(h, tq, tk, st):
                    def f():
                        r = tk - 4 * tq
                        diag = r >= 0
                        c0 = r * 128 if diag else 0
                        cr = slice(c0, TQ)
                        sps = sc_ps.tile([128, TQ], F32, tag="sc",
                                         name=f"sps_{h}_{tq}_{tk}")
                        nc.tensor.matmul(
                            sps[:, cr],
                            qkt_sb[HPC + h][tk // 4][:, (tk % 4) * 128:(tk % 4 + 1) * 128],
                            qkt_sb[h][tq][:, cr],
                            start=True, stop=True, skip_group_check=True,
                        )
                        es = es_p.tile([128, TQ], F16, tag="es",
                                       name=f"es_{h}_{tq}_{tk}")
                        nc.scalar.activation(es[:, cr], sps[:, cr], EXP, scale=SCALE)
                        if diag:
                            nc.vector.tensor_tensor(
                                es[:, c0:c0 + 128], es[:, c0:c0 + 128],
                                mask_sb[:], op=AluOpType.mult,
                            )
                        st["es"][tk] = es
                    return (int(216 * (TQ - (max(tk - 4 * tq, 0)) * 128) / TQ), f)

                def blk_b(h, tq, tk, st):
                    def f():
                        r = tk - 4 * tq
                        diag = r >= 0
                        c0 = r * 128 if diag else 0
                        cr = slice(c0, TQ)
                        if tk == 0:
                            st["yps"] = y_ps.tile([128, TQ], F32, tag="y",
                                                  name=f"yps_{h}_{tq}")
                        es = st["es"][tk]
                        nc.tensor.matmul(
                            st["yps"][:, cr], v_sb[tk][:, h * 128:(h + 1) * 128],
                            es[:, cr],
                            start=(tk == 0), stop=(tk == 4 * tq + 3),
                            skip_group_check=True,
                        )
                        # denominator bookkeeping: full-width tiles pair up in
                        # fp16 (DVE 4x); ragged diagonals accumulate at the end
                        if not diag or r == 0:
                            st["fulls"].append(es)
                            if len(st["fulls"]) >= 2:
                                a = st["fulls"].pop(0)
                                b = st["fulls"].pop(0)
                                pr = pair_p.tile([128, TQ], F16, tag="pr",
                                                 name=f"pr_{h}_{tq}_{tk}")
                                nc.vector.tensor_tensor(pr[:], a[:], b[:],
                                                        op=AluOpType.add)
                                st["fulls"].append(pr)
                        else:
                            st["rag"].append((es, c0))
                    return (int(216 * (TQ - (max(tk - 4 * tq, 0)) * 128) / TQ), f)

                def denom(h, tq, st):
                    def f():
                        dacc = st["fulls"][0]
                        for es, c0 in st["rag"]:
                            nc.vector.tensor_tensor(
                                dacc[:, c0:], dacc[:, c0:], es[:, c0:],
                                op=AluOpType.add,
                            )
                        # ones[128,128] @ dacc = column sums replicated on all
                        # partitions: reduction + broadcast in one matmul
                        dbc = sc_ps.tile([128, TQ], F32, tag="sc",
                                         name=f"dbc_{h}_{tq}")
                        nc.tensor.matmul(dbc[:], onem[:], dacc[:],
                                         start=True, stop=True,
                                         skip_group_check=True)
                        rcp = dn_p.tile([128, TQ], F32, tag="rcp",
                                        name=f"rcp_{h}_{tq}")
                        nc.vector.reciprocal_approx_fast(rcp[:], dbc[:])
                        nc.vector.tensor_tensor(
                            yt_sb[h][tq][:], st["yps"][:], rcp[:],
                            op=AluOpType.mult,
                        )
                    return (300, f)

                head_units.append(
                    ([blk_a(h, tq, tk, st) for tk in range(nblk)],
                     [blk_b(h, tq, tk, st) for tk in range(nblk)],
                     denom(h, tq, st)))
            # continuous scores/AV pipeline across all heads of this tile at
            # depth 3: the next head's scores provide PE spacing for this
            # head's tail AV units; denominators trail their head by 2 units
            A = [u for a, b, dn in head_units for u in a]
            Bv = [u for a, b, dn in head_units for u in b]
            depth = 3
            pend = []

            def push(u):
                units.append(u)
                for p in pend:
                    p[0] -= 1
                while pend and pend[0][0] <= 0:
                    units.append(pend.pop(0)[1])

            for k in range(len(A) + depth):
                if k < len(A):
                    push(A[k])
                if k >= depth:
                    push(Bv[k - depth])
                    if (k - depth) % nblk == nblk - 1:
                        pend.append([2, head_units[(k - depth) // nblk][2]])
            while pend:
                units.append(pend.pop(0)[1])
            return units

        pools = {}

        def proj_unit(tqb, nb):
            def f():
                ps = mm_ps.tile([128, TQ], F32, tag="mm", name=f"ops_{tqb}_{nb}")
                for h in range(HPC):
                    nc.tensor.matmul(
                        ps[:],
                        yt_sb[h][tqb // 4][:, (tqb % 4) * 128:(tqb % 4 + 1) * 128],
                        wo_sb[h][:, nb * TQ:(nb + 1) * TQ],
                        start=(h == 0), stop=(h == HPC - 1),
                        skip_group_check=True,
                    )
                osb = pools["o_ev"].tile([128, TQ], F16, tag="osb",
                                         name=f"osb_{tqb}_{nb}")
                if nb % 2 == 0:
                    nc.vector.tensor_copy(osb[:], ps[:])
                else:
                    nc.scalar.copy(osb[:], ps[:])
                nc.sync.dma_start(
                    out[tqb * 128:(tqb + 1) * 128, nb * TQ:(nb + 1) * TQ], osb[:]
                )
            return (4 * 216, f)

        # ---------------- weave + emit ----------------
        def weave(a, b, bias=2000, lead=2):
            # a = filler stream (phase-1/proj), b = latency-sensitive stream
            # (sdpa); bias keeps b ahead so a covers the segment tail; lead
            # filler units up front cover the first exp of the segment
            ta = sum(w for w, _ in a) or 1
            tb = sum(w for w, _ in b) or 1
            ca = cb = 0
            i = j = 0
            while i < min(lead, len(a)):
                ca += a[i][0]
                a[i][1]()
                i += 1
            while i < len(a) or j < len(b):
                if j >= len(b) or (i < len(a) and ca * tb <= max(cb - bias, 0) * ta):
                    ca += a[i][0]
                    a[i][1]()
                    i += 1
                else:
                    cb += b[j][0]
                    b[j][1]()
                    j += 1

        for idx, (_, f) in enumerate(p1_units(0)):
            f()
            if idx == 0:
                # wv (2MB, needed ~27us later by the V units) trails the
                # scalar queue's startup transfers; slab 1 trails wv
                nc.scalar.dma_start(wv_all[:], wv[:])
            elif idx == 1:
                dma_slab(1)
        for g in range(1, NTQ):
            # slab g was prefetched a segment ago; fetch g+1 now (the xt
            # pool's 2-deep ring gives the WAR gating: slab g+1 overwrites
            # slab g-1's buffer, whose readers finished last segment)
            if g + 1 < NTQ:
                dma_slab(g + 1)
            if g == 2:
                for h in range(HPC):
                    nc.scalar.dma_start(wo_sb[h][:], wout[h])
            weave(p1_units(g), sdpa_units(g - 1))
        p1_stack.close()
        with tc.tile_pool(name="o_ev", bufs=4) as o_ev:
            pools["o_ev"] = o_ev
            # all 48 tile-0..2 units weave into sdpa(3) so their output
            # DMAs drain during the segment; only the 16 tile-3-dependent
            # units trail, their DMAs pipelining behind the PE stream
            proj = [proj_unit(tqb, nb) for tqb in range(12) for nb in range(4)]
            weave(proj, sdpa_units(NTQ - 1))
            for tqb in range(12, 16):
                for nb in range(4):
                    proj_unit(tqb, nb)[1]()

    nc.compile()
    return nc


def _host_tables():
    positions = np.arange(T, dtype=np.float64)
    inv_freq = 1.0 / (THETA ** (np.arange(0, D, 2, dtype=np.float64) / D))
    freqs = np.outer(positions, inv_freq)          # [T, 64]
    cs = np.concatenate([np.cos(freqs).T, np.sin(freqs).T]).astype(np.float16)   # [128, T]
    p = np.arange(128)[:, None]
    j = np.arange(128)[None, :]
    mask = (p <= j).astype(np.float16)             # [128, 128] lower-tri visibility
    return cs, mask


def kernel(x, W_qkv, W_out):
    global _compiled
    if _compiled is None:
        _compiled = _build()
    nc = _compiled

    x = np.ascontiguousarray(np.asarray(x, dtype=np.float32))
    W_qkv = np.asarray(W_qkv, dtype=np.float32)
    W_out = np.asarray(W_out, dtype=np.float32)

    cs, mask = _host_tables()
    ones_mm = np.ones((128, 128), np.float16)

    perm = np.concatenate([np.arange(0, D, 2), np.arange(1, D, 2)])  # de-interleave

    in_maps = []
    for c in range(N_CORES):
        b, tp = divmod(c, TP)
        heads = np.arange(tp * HPC, (tp + 1) * HPC)
        qk_cols = np.concatenate(
            [h * D + perm for h in heads] + [E + h * D + perm for h in heads]
        )
        v_cols = np.concatenate([2 * E + h * D + np.arange(D) for h in heads])
        # wqk f8-major: [f8, p, e*128+m]
        wqk_l = np.ascontiguousarray(
            W_qkv[:, qk_cols].reshape(EC, 128, 2 * HPC, 128)
            .transpose(2, 1, 0, 3).reshape(2 * HPC, 128, EC * 128)
        )
        # wv e-major along free dim: [p, e*FL+m]
        wv_l = np.ascontiguousarray(
            W_qkv[:, v_cols].reshape(EC, 128, FL)
            .transpose(1, 0, 2).reshape(128, EC * FL)
        )
        wout_l = np.ascontiguousarray(
            W_out.reshape(N_HEAD, D, E)[heads].reshape(HPC, 128, E)
        )
        # x slab: [t4, p, e*TQ+t]
        xt4 = np.ascontiguousarray(
            x[b].reshape(NTQ, TQ, EC, 128).transpose(0, 3, 2, 1)
            .reshape(NTQ, 128, EC * TQ)
        ).astype(np.float16)
        in_maps.append({
            "xT": xt4,
            "wqk": wqk_l.astype(np.float16),
            "wv": wv_l.astype(np.float16),
            "wout": wout_l.astype(np.float16),
            "csx": cs,
            "mask1": mask,
            "ones_m": ones_mm,
        })

    global _last_in_maps
    _last_in_maps = in_maps
    res = bass_utils.run_bass_kernel_spmd(nc, in_maps, core_ids=list(range(N_CORES)))
    out = np.zeros((B, T, E), dtype=np.float32)
    for c in range(N_CORES):
        out[c // TP] += res.results[c]["out"]
    return out



# revision 53
# speedup vs baseline: 1.0030x; 1.0030x over previous
"""Trainium2 Bass kernel for a causal attention block (B=2, T=2048, E=2048,
16 heads, head_dim=128, interleaved RoPE).

Sharding: data-parallel over batch (2) x tensor-parallel over heads (4 per
core) = 8 NeuronCores. Each core computes QKV projection for its 4 heads,
RoPE, causal SDPA, and a partial output projection (row-sharded W_out); the
host sums the 4 TP partials per batch element.

Single software-pipelined instruction stream (fp16 matmuls, fp32 PSUM):
  - warmup matmuls on a memset tile ramp the PE p-state while the first
    DMAs land (the clock drops back to mid p-state after any PE idle, so
    startup stalls cost ~2x their face value).
  - startup transfers are striped across the sync and scalar trigger
    queues (~150-190GB/s each, ~380GB/s aggregate; the gpsimd queue is
    25x slower) in exact consumption order, and the first two QK units
    run block-staggered chunk-interleaved so each freshly-landed x chunk
    feeds 1024 output rows — the PE consumption rate matches the DMA
    supply rate instead of outrunning it 2:1.
  - segment g in 0..3 runs the QKV projection for token slab g (QT/KT
    transposed + RoPE'd on eviction, V natural) interleaved with SDPA for
    query tile g-1; causality means tile g-1 only needs slabs <= g-1, and
    the dependency-free projection matmuls keep the PE busy while ACT/DVE
    drain the softmax chain.  Slabs prefetch a full segment early, split
    across both fast queues.
  - SDPA diagonal blocks are column-restricted (queries below the diagonal
    are never computed), the softmax denominator tree runs in fp16 (DVE 4x
    mode) + a ones-matmul partition reduction, and the output projection
    for query tiles 0..2 interleaves into the tq=3 SDPA round so its
    output DMAs drain during compute.
fp8 was evaluated and rejected: DoubleRow matmuls measure 2x fp16 on
real hw (not the cost model's 4x), so the accuracy-preserving hi/lo
3-pass split is slower than fp16, and raw 1-pass fp8 fails the 2e-2
gate (4.4e-2 measured in simulation).
"""

import sys
from contextlib import ExitStack

sys.path.insert(0, "/opt/trn_rl_repo")

import numpy as np

import bass_rust
import concourse.bacc as bacc
import concourse.mybir as mybir
from concourse.alu_op_type import AluOpType
from concourse import tile
from concourse import bass_utils

B, T, E = 2, 2048, 2048
N_HEAD = 16
D = E // N_HEAD            # 128
THETA = 10000.0
N_CORES = 8
TP = 4                     # tensor-parallel degree (heads)
HPC = N_HEAD // TP         # heads per core = 4
FL = HPC * D               # local head width = 512
EC = E // 128              # 16 contraction chunks
TQ = 512                   # query tile (free dim)
NTQ = T // TQ              # 4
NTK = T // 128             # 16
N_WARM = 17                # PE p-state warmup matmuls

F32 = mybir.dt.float32
F32R = mybir.dt.float32r
F16 = mybir.dt.float16
EXP = mybir.ActivationFunctionType.Exp
SCALE = 1.0 / np.sqrt(D)

_compiled = None
_last_in_maps = None


def _build():
    nc = bacc.Bacc("TRN2", target_bir_lowering=False)

    xT = nc.dram_tensor("xT", (NTQ, 128, EC * TQ), F16, kind="ExternalInput")
    wqk = nc.dram_tensor("wqk", (2 * HPC, 128, EC * 128), F16, kind="ExternalInput")
    wv = nc.dram_tensor("wv", (128, EC * FL), F16, kind="ExternalInput")
    wout = nc.dram_tensor("wout", (HPC, 128, E), F16, kind="ExternalInput")
    csx = nc.dram_tensor("csx", (128, T), F16, kind="ExternalInput")
    mask1 = nc.dram_tensor("mask1", (128, 128), F16, kind="ExternalInput")
    ones_m = nc.dram_tensor("ones_m", (128, 128), F16, kind="ExternalInput")
    out = nc.dram_tensor("out", (T, E), F16, kind="ExternalOutput")

    with tile.TileContext(nc) as tc, nc.allow_low_precision(
        reason="fp16 matmul inputs / fp16 softmax stats are intentional"
    ), tc.tile_pool(name="const", bufs=1) as const, \
         tc.tile_pool(name="wo_p", bufs=1) as wo_p, \
         tc.tile_pool(name="qkt_p", bufs=1) as qkt_p, \
         tc.tile_pool(name="v_p", bufs=1) as v_p, \
         tc.tile_pool(name="yt_p", bufs=1) as yt_p, \
         tc.tile_pool(name="es_p", bufs=12) as es_p, \
         tc.tile_pool(name="pair_p", bufs=4) as pair_p, \
         tc.tile_pool(name="dn_p", bufs=1) as dn_p, \
         tc.tile_pool(name="mm_ps", bufs=2, space="PSUM") as mm_ps, \
         tc.tile_pool(name="sc_ps", bufs=4, space="PSUM") as sc_ps, \
         tc.tile_pool(name="y_ps", bufs=2, space="PSUM") as y_ps:

        # phase-1 pools: closed before the final projection segment to free
        # SBUF for the output-eviction pool (stack allocation is LIFO)
        p1_stack = ExitStack()
        wqk_p = p1_stack.enter_context(tc.tile_pool(name="wqk_p", bufs=1))
        wv_p = p1_stack.enter_context(tc.tile_pool(name="wv_p", bufs=1))
        xt_p = p1_stack.enter_context(tc.tile_pool(name="xt_p", bufs=2))
        rope_t = p1_stack.enter_context(tc.tile_pool(name="rope_t", bufs=3))

        cs_sb = const.tile([128, T], F16, tag="cs")    # [cos; sin]
        csd_sb = const.tile([128, T], F16, tag="csd")  # [sin; cos]
        mask_sb = const.tile([128, 128], F16, tag="mask")
        onem = const.tile([128, 128], F16, tag="onem")
        warm_sb = const.tile([128, TQ], F16, tag="warm")

        # wqk stored f8-major: one [128, EC*128] tile per 128-wide qk block so
        # the first projection unit is gated by 0.5MB of weights, not 4MB
        wqk_sb = [wqk_p.tile([128, EC * 128], F16, tag=f"wqk{f}", name=f"wqk_sb{f}")
                  for f in range(2 * HPC)]
        wv_all = wv_p.tile([128, EC * FL], F16, tag="wv")
        wo_sb = [wo_p.tile([128, E], F16, tag=f"wo{h}", name=f"wo_sb{h}")
                 for h in range(HPC)]

        # resident intermediates: QT/KT (transposed, de-interleaved, RoPE'd),
        # V (natural layout), normalized attention outputs
        qkt_sb = [[qkt_p.tile([128, TQ], F16, tag=f"qkt{f}_{t}", name=f"qkt_sb{f}_{t}")
                   for t in range(NTQ)] for f in range(2 * HPC)]
        v_sb = [v_p.tile([128, FL], F16, tag=f"v{t}", name=f"v_sb{t}")
                for t in range(NTK)]
        yt_sb = [[yt_p.tile([128, TQ], F16, tag=f"yt{h}_{t}", name=f"yt_sb{h}_{t}")
                  for t in range(NTQ)] for h in range(HPC)]

        # ---------------- DMA + warmup ----------------
        xt_sb = {}

        def dma_slab(t4):
            # halves ride both fast queues: the trigger is WAR-gated on the
            # xt ring, so when it finally fires the transfer must finish
            # quickly (a single-queue 2MB transfer stalls the first reader)
            xt = xt_p.tile([128, EC * TQ], F16, tag="xt", name=f"xt_{t4}")
            half = EC * TQ // 2
            nc.sync.dma_start(xt[:, 0:half], xT[t4, :, 0:half])
            nc.scalar.dma_start(xt[:, half:], xT[t4, :, half:])
            xt_sb[t4] = xt

        # PE p-state warmup on a memset tile (no DMA dependency)
        nc.gpsimd.memset(warm_sb[:], 1.0)

        # startup DMA: slab 0 rides the sync queue as 4-chunk group DMAs
        # (4KB/partition lines; 16 small transfers would pay ~600ns trigger
        # cost each and the queue's ~190GB/s issue rate); weights ride the
        # scalar queue in parallel (the 16 shared DMA engines give
        # ~380GB/s aggregate); slab 1 prefetches behind slab 0.
        # NB: the gpsimd-triggered DMA queue moves ~1.5GB/s/engine (25x
        # slower than sync/scalar queues) — only tiny constants go there.
        xt0 = xt_p.tile([128, EC * TQ], F16, tag="xt", name="xt_0")
        xt_sb[0] = xt0

        # each fast queue serializes its transfers at ~150-190GB/s from
        # ~8.6us, so arrivals are predictable: stripe x groups and weights
        # across sync/scalar in exact pair-consumption order.  The pair
        # unit consumes a fresh 4-chunk group every ~1.7us, matching the
        # striped supply, so the PE never stalls (and never drops p-state).
        # sync:   g0, wqk1, g2, cs, wqk4-7, slab1a
        # scalar: wqk0, g1, g3, wqk2, wqk3, (inj:) wv, slab1b
        def xgrp(e4):
            return (xt0[:, e4 * TQ:(e4 + 4) * TQ],
                    xT[0, :, e4 * TQ:(e4 + 4) * TQ])

        nc.sync.dma_start(*xgrp(0))
        nc.sync.dma_start(wqk_sb[1][:], wqk[1])
        nc.sync.dma_start(*xgrp(8))
        nc.sync.dma_start(wqk_sb[2][:], wqk[2])
        nc.sync.dma_start(cs_sb[:], csx[:])
        for f in (6, 7):
            nc.sync.dma_start(wqk_sb[f][:], wqk[f])
        nc.scalar.dma_start(wqk_sb[0][:], wqk[0])
        nc.scalar.dma_start(*xgrp(4))
        nc.scalar.dma_start(*xgrp(12))
        nc.scalar.dma_start(wqk_sb[3][:], wqk[3])
        # wqk4/5 ride scalar (idle after wqk3) so the mid-phase weights
        # all arrive with ~3us of margin instead of racing consumption
        nc.scalar.dma_start(wqk_sb[4][:], wqk[4])
        nc.scalar.dma_start(wqk_sb[5][:], wqk[5])
        # preload the ACT function tables (1.3us) during the DMA window so
        # the first qk16 eviction isn't serialized behind the table load
        # (after the triggers: the scalar engine must not delay them)
        actw = const.tile([128, 8], F16, tag="actw")
        nc.scalar.activation(actw[:], warm_sb[:, 0:8], EXP, scale=0.125)
        nc.scalar.copy(actw[:], warm_sb[:, 0:8])
        nc.gpsimd.dma_start(mask_sb[:], mask1[:])
        nc.gpsimd.dma_start(onem[:], ones_m[:])
        # csd = [sin; cos] is cs with halves swapped: derive on-chip
        nc.vector.tensor_copy(csd_sb[0:64, :], cs_sb[64:128, :])
        nc.vector.tensor_copy(csd_sb[64:128, :], cs_sb[0:64, :])
        wps = sc_ps.tile([128, TQ], F32, tag="sc", name="warm_ps")
        for i in range(N_WARM):
            nc.tensor.matmul(wps[:], warm_sb[:, 0:128], warm_sb[:],
                             start=True, stop=True, skip_group_check=True)

        # ---------------- unit builders ----------------
        qk16_sb = {}

        def qk_unit_mm(t4, f8):
            def f():
                ps = mm_ps.tile([128, TQ], F32, tag="mm", name=f"qkps_{t4}_{f8}")
                for e in range(EC):
                    nc.tensor.matmul(
                        ps[:], wqk_sb[f8][:, e * 128:(e + 1) * 128],
                        xt_sb[t4][:, e * TQ:(e + 1) * TQ],
                        start=(e == 0), stop=(e == EC - 1),
                        skip_group_check=True,
                    )
                qk16 = rope_t.tile([128, TQ], F16, tag="qk16",
                                   name=f"qk16_{t4}_{f8}")
                nc.scalar.copy(qk16[:], ps[:])
                qk16_sb[(t4, f8)] = qk16
            return (16 * 216, f)

        def qk_pair_mm(t4):
            # units f8=0,1 interleaved chunk-wise: each freshly-DMA'd x
            # chunk feeds 1024 output rows instead of 512, halving the
            # startup demand rate so the slab-0 transfers keep pace with
            # the PE (a stalled PE also drops back to mid p-state, so
            # startup stalls cost ~2x their face value).  psb lives in the
            # sc_ps ring so the next unit's mm psum has no WAR on us.
            def f():
                psa = mm_ps.tile([128, TQ], F32, tag="mm", name=f"qkps_{t4}_0")
                psb = sc_ps.tile([128, TQ], F32, tag="sc", name=f"qkps_{t4}_1")

                def mm(ps, f8, e):
                    nc.tensor.matmul(
                        ps[:], wqk_sb[f8][:, e * 128:(e + 1) * 128],
                        xt_sb[t4][:, e * TQ:(e + 1) * TQ],
                        start=(e == 0), stop=(e == EC - 1),
                        skip_group_check=True,
                    )

                # block-staggered: b trails a by one 4-chunk group, so
                # wqk1 is needed ~0.85us after start and each fresh group
                # feeds 8 matmuls (a once, b once)
                for e4 in range(0, EC, 4):
                    for e in range(e4, e4 + 4):
                        mm(psa, 0, e)
                    if e4 == EC - 4:
                        qk16a = rope_t.tile([128, TQ], F16, tag="qk16",
                                            name=f"qk16_{t4}_0")
                        nc.scalar.copy(qk16a[:], psa[:])
                        qk16_sb[(t4, 0)] = qk16a
                    for e in range(e4, e4 + 4):
                        mm(psb, 1, e)
                qk16b = rope_t.tile([128, TQ], F16, tag="qk16",
                                    name=f"qk16_{t4}_1")
                nc.scalar.copy(qk16b[:], psb[:])
                qk16_sb[(t4, 1)] = qk16b
            return (2 * 16 * 216, f)

        def qk_unit_rope(t4, f8):
            # separate weave unit: keeps the DVE queue free for sdpa masks
            # between rope batches (rope results are only needed next segment)
            def f():
                ts4 = slice(t4 * TQ, (t4 + 1) * TQ)
                dst = qkt_sb[f8][t4]
                qk16 = qk16_sb[(t4, f8)]
                t_a = rope_t.tile([64, TQ], F16, tag="ta")
                t_b = rope_t.tile([64, TQ], F16, tag="tb")
                nc.vector.tensor_tensor(t_a[:], qk16[0:64, :], cs_sb[0:64, ts4], op=AluOpType.mult)
                nc.vector.tensor_tensor(t_b[:], qk16[64:128, :], cs_sb[64:128, ts4], op=AluOpType.mult)
                nc.vector.tensor_tensor(dst[0:64, :], t_a[:], t_b[:], op=AluOpType.subtract)
                t_c = rope_t.tile([64, TQ], F16, tag="tc")
                t_d = rope_t.tile([64, TQ], F16, tag="td")
                nc.vector.tensor_tensor(t_c[:], qk16[0:64, :], csd_sb[0:64, ts4], op=AluOpType.mult)
                nc.vector.tensor_tensor(t_d[:], qk16[64:128, :], csd_sb[64:128, ts4], op=AluOpType.mult)
                nc.vector.tensor_tensor(dst[64:128, :], t_c[:], t_d[:], op=AluOpType.add)
            return (300, f)

        def v_unit(t4, i):
            def f():
                tk = 4 * t4 + i
                ps = mm_ps.tile([128, FL], F32, tag="mm", name=f"vps_{tk}")
                for e in range(EC):
                    nc.tensor.matmul(
                        ps[:], xt_sb[t4][:, e * TQ + i * 128:e * TQ + (i + 1) * 128],
                        wv_all[:, e * FL:(e + 1) * FL],
                        start=(e == 0), stop=(e == EC - 1),
                        skip_group_check=True,
                    )
                nc.scalar.copy(v_sb[tk][:], ps[:])
            return (16 * 216, f)

        def p1_units(t4):
            if t4 == 0:
                mm = [qk_pair_mm(0)] + \
                     [qk_unit_mm(0, f8) for f8 in range(2, 2 * HPC)] + \
                     [v_unit(0, i) for i in range(4)]
            else:
                mm = [qk_unit_mm(t4, f8) for f8 in range(2 * HPC)] + \
                     [v_unit(t4, i) for i in range(4)]
            rp = [qk_unit_rope(t4, f8) for f8 in range(2 * HPC)]
            units = []
            for i, u in enumerate(mm):
                units.append(u)
                if i >= 1 and i - 1 < len(rp):
                    units.append(rp[i - 1])
            units.extend(rp[len(mm) - 1:])
            return units

        def sdpa_units(tq):
            units = []
            head_units = []
            for h in range(HPC):
                st = {"fulls": [], "rag": [], "yps": None, "es": {}}
                nblk = 4 * tq + 4

                def blk_a(h, tq, tk, st):
                    def f():
                        r = tk - 4 * tq
                        diag = r >= 0
                        c0 = r * 128 if diag else 0
                        cr = slice(c0, TQ)
                        sps = sc_ps.tile([128, TQ], F32, tag="sc",
                                         name=f"sps_{h}_{tq}_{tk}")
                        nc.tensor.matmul(
                            sps[:, cr],
                            qkt_sb[HPC + h][tk // 4][:, (tk % 4) * 128:(tk % 4 + 1) * 128],
                            qkt_sb[h][tq][:, cr],
                            start=True, stop=True, skip_group_check=True,
                        )
                        es = es_p.tile([128, TQ], F16, tag="es",
                                       name=f"es_{h}_{tq}_{tk}")
                        nc.scalar.activation(es[:, cr], sps[:, cr], EXP, scale=SCALE)
                        if diag:
                            nc.vector.tensor_tensor(
                                es[:, c0:c0 + 128], es[:, c0:c0 + 128],
                                mask_sb[:], op=AluOpType.mult,
                            )
                        st["es"][tk] = es
                    return (int(216 * (TQ - (max(tk - 4 * tq, 0)) * 128) / TQ), f)

                def blk_b(h, tq, tk, st):
                    def f():
                        r = tk - 4 * tq
                        diag = r >= 0
                        c0 = r * 128 if diag else 0
                        cr = slice(c0, TQ)
                        if tk == 0:
                            st["yps"] = y_ps.tile([128, TQ], F32, tag="y",
                                                  name=f"yps_{h}_{tq}")
                        es = st["es"][tk]
                        nc.tensor.matmul(
                            st["yps"][:, cr], v_sb[tk][:, h * 128:(h + 1) * 128],
                            es[:, cr],
                            start=(tk == 0), stop=(tk == 4 * tq + 3),
                            skip_group_check=True,
                        )
                        # denominator bookkeeping: full-width tiles pair up in
                        # fp16 (DVE 4x); ragged diagonals accumulate at the end
                        if not diag or r == 0:
                            st["fulls"].append(es)
                            if len(st["fulls"]) >= 2:
                                a = st["fulls"].pop(0)
                                b = st["fulls"].pop(0)
                                pr = pair_p.tile([128, TQ], F16, tag="pr",
                                                 name=f"pr_{h}_{tq}_{tk}")
                                nc.vector.tensor_tensor(pr[:], a[:], b[:],
                                                        op=AluOpType.add)
                                st["fulls"].append(pr)
                        else:
                            st["rag"].append((es, c0))
                    return (int(216 * (TQ - (max(tk - 4 * tq, 0)) * 128) / TQ), f)

                def denom(h, tq, st):
                    def f():
                        dacc = st["fulls"][0]
                        for es, c0 in st["rag"]:
                            nc.vector.tensor_tensor(
                                dacc[:, c0:], dacc[:, c0:], es[:, c0:],
                                op=AluOpType.add,
                            )
                        # ones[128,128] @ dacc = column sums replicated on all
                        # partitions: reduction + broadcast in one matmul
                        dbc = sc_ps.tile([128, TQ], F32, tag="sc",
                                         name=f"dbc_{h}_{tq}")
                        nc.tensor.matmul(dbc[:], onem[:], dacc[:],
                                         start=True, stop=True,
                                         skip_group_check=True)
                        rcp = dn_p.tile([128, TQ], F32, tag="rcp",
                                        name=f"rcp_{h}_{tq}")
                        nc.vector.reciprocal_approx_fast(rcp[:], dbc[:])
                        nc.vector.tensor_tensor(
                            yt_sb[h][tq][:], st["yps"][:], rcp[:],
                            op=AluOpType.mult,
                        )
                    return (300, f)

                head_units.append(
                    ([blk_a(h, tq, tk, st) for tk in range(nblk)],
                     [blk_b(h, tq, tk, st) for tk in range(nblk)],
                     denom(h, tq, st)))
            # continuous scores/AV pipeline across all heads of this tile at
            # depth 3: the next head's scores provide PE spacing for this
            # head's tail AV units; denominators trail their head by 2 units
            A = [u for a, b, dn in head_units for u in a]
            Bv = [u for a, b, dn in head_units for u in b]
            depth = 3
            pend = []

            def push(u):
                units.append(u)
                for p in pend:
                    p[0] -= 1
                while pend and pend[0][0] <= 0:
                    units.append(pend.pop(0)[1])

            for k in range(len(A) + depth):
                if k < len(A):
                    push(A[k])
                if k >= depth:
                    push(Bv[k - depth])
                    if (k - depth) % nblk == nblk - 1:
                        pend.append([2, head_units[(k - depth) // nblk][2]])
            while pend:
                units.append(pend.pop(0)[1])
            return units

        pools = {}

        def proj_unit(tqb, nb):
            def f():
                ps = mm_ps.tile([128, TQ], F32, tag="mm", name=f"ops_{tqb}_{nb}")
                for h in range(HPC):
                    nc.tensor.matmul(
                        ps[:],
                        yt_sb[h][tqb // 4][:, (tqb % 4) * 128:(tqb % 4 + 1) * 128],
                        wo_sb[h][:, nb * TQ:(nb + 1) * TQ],
                        start=(h == 0), stop=(h == HPC - 1),
                        skip_group_check=True,
                    )
                osb = pools["o_ev"].tile([128, TQ], F16, tag="osb",
                                         name=f"osb_{tqb}_{nb}")
                if nb % 2 == 0:
                    nc.vector.tensor_copy(osb[:], ps[:])
                else:
                    nc.scalar.copy(osb[:], ps[:])
                nc.sync.dma_start(
                    out[tqb * 128:(tqb + 1) * 128, nb * TQ:(nb + 1) * TQ], osb[:]
                )
            return (4 * 216, f)

        # ---------------- weave + emit ----------------
        def weave(a, b, bias=2000, lead=2):
            # a = filler stream (phase-1/proj), b = latency-sensitive stream
            # (sdpa); bias keeps b ahead so a covers the segment tail; lead
            # filler units up front cover the first exp of the segment
            ta = sum(w for w, _ in a) or 1
            tb = sum(w for w, _ in b) or 1
            ca = cb = 0
            i = j = 0
            while i < min(lead, len(a)):
                ca += a[i][0]
                a[i][1]()
                i += 1
            while i < len(a) or j < len(b):
                if j >= len(b) or (i < len(a) and ca * tb <= max(cb - bias, 0) * ta):
                    ca += a[i][0]
                    a[i][1]()
                    i += 1
                else:
                    cb += b[j][0]
                    b[j][1]()
                    j += 1

        for idx, (_, f) in enumerate(p1_units(0)):
            f()
            if idx == 0:
                # wv (2MB, needed ~27us later by the V units) trails the
                # scalar queue's startup transfers; slab 1 trails wv
                nc.scalar.dma_start(wv_all[:], wv[:])
            elif idx == 1:
                dma_slab(1)
        for g in range(1, NTQ):
            # slab g was prefetched a segment ago; fetch g+1 now (the xt
            # pool's 2-deep ring gives the WAR gating: slab g+1 overwrites
            # slab g-1's buffer, whose readers finished last segment)
            if g + 1 < NTQ:
                dma_slab(g + 1)
            if g == 2:
                for h in range(HPC):
                    nc.scalar.dma_start(wo_sb[h][:], wout[h])
            weave(p1_units(g), sdpa_units(g - 1))
        p1_stack.close()
        with tc.tile_pool(name="o_ev", bufs=4) as o_ev:
            pools["o_ev"] = o_ev
            # all 48 tile-0..2 units weave into sdpa(3) so their output
            # DMAs drain during the segment; only the 16 tile-3-dependent
            # units trail, their DMAs pipelining behind the PE stream
            proj = [proj_unit(tqb, nb) for tqb in range(12) for nb in range(4)]
            weave(proj, sdpa_units(NTQ - 1))
            for tqb in range(12, 16):
                for nb in range(4):
                    proj_unit(tqb, nb)[1]()

    nc.compile()
    return nc


def _host_tables():
    positions = np.arange(T, dtype=np.float64)
    inv_freq = 1.0 / (THETA ** (np.arange(0, D, 2, dtype=np.float64) / D))
    freqs = np.outer(positions, inv_freq)          # [T, 64]
    cs = np.concatenate([np.cos(freqs).T, np.sin(freqs).T]).astype(np.float16)   # [128, T]
    p = np.arange(128)[:, None]
    j = np.arange(128)[None, :]
    mask = (p <= j).astype(np.float16)             # [128, 128] lower-tri visibility
    return cs, mask


def kernel(x, W_qkv, W_out):
    global _compiled
    if _compiled is None:
        _compiled = _build()
    nc = _compiled

    x = np.ascontiguousarray(np.asarray(x, dtype=np.float32))
    W_qkv = np.asarray(W_qkv, dtype=np.float32)
    W_out = np.asarray(W_out, dtype=np.float32)

    cs, mask = _host_tables()
    ones_mm = np.ones((128, 128), np.float16)

    perm = np.concatenate([np.arange(0, D, 2), np.arange(1, D, 2)])  # de-interleave

    in_maps = []
    for c in range(N_CORES):
        b, tp = divmod(c, TP)
        heads = np.arange(tp * HPC, (tp + 1) * HPC)
        qk_cols = np.concatenate(
            [h * D + perm for h in heads] + [E + h * D + perm for h in heads]
        )
        v_cols = np.concatenate([2 * E + h * D + np.arange(D) for h in heads])
        # wqk f8-major: [f8, p, e*128+m]
        wqk_l = np.ascontiguousarray(
            W_qkv[:, qk_cols].reshape(EC, 128, 2 * HPC, 128)
            .transpose(2, 1, 0, 3).reshape(2 * HPC, 128, EC * 128)
        )
        # wv e-major along free dim: [p, e*FL+m]
        wv_l = np.ascontiguousarray(
            W_qkv[:, v_cols].reshape(EC, 128, FL)
            .transpose(1, 0, 2).reshape(128, EC * FL)
        )
        wout_l = np.ascontiguousarray(
            W_out.reshape(N_HEAD, D, E)[heads].reshape(HPC, 128, E)
        )
        # x slab: [t4, p, e*TQ+t]
        xt4 = np.ascontiguousarray(
            x[b].reshape(NTQ, TQ, EC, 128).transpose(0, 3, 2, 1)
            .reshape(NTQ, 128, EC * TQ)
        ).astype(np.float16)
        in_maps.append({
            "xT": xt4,
            "wqk": wqk_l.astype(np.float16),
            "wv": wv_l.astype(np.float16),
            "wout": wout_l.astype(np.float16),
            "csx": cs,
            "mask1": mask,
            "ones_m": ones_mm,
        })

    global _last_in_maps
    _last_in_maps = in_maps
    res = bass_utils.run_bass_kernel_spmd(nc, in_maps, core_ids=list(range(N_CORES)))
    out = np.zeros((B, T, E), dtype=np.float32)
    for c in range(N_CORES):
        out[c // TP] += res.results[c]["out"]
    return out

